# revision 10
# baseline (speedup 1.0000x reference)
"""BLSTM Trainium2 kernel: 8-core SPMD, SBUF-resident wavefront.

Core pair q={2q,2q+1} owns batch element q; even core runs the forward
2-layer LSTM chain, odd the backward chain (host feeds time-reversed
frames). Both layers are software-pipelined (L1 lags L0 by 2 steps) on
one core; Wx@x_t and the bias are fused into each step's gate PSUM
accumulation, so nothing round-trips through DRAM. The final-layer
hidden stream stays in SBUF (a dynamic-offset copy un-reverses time on
odd cores), is projected per-frame through this core's half of Wp, and
overlap-added into an fp16 accumulator seeded with skip+bias. The pair
exchanges projected partials; both cores emit the identical full
[512, 4200] output (host reads the even core's).
"""
import numpy as np
from contextlib import ExitStack

U = 512
S = 200          # frame width (LSTM steps)
F = 41           # frames per batch element
T = 4200
STRIDE = 100
COLS = S * F     # 8200 time-major columns per core (col = s*41 + f)
G = 4 * U        # 2048 gate rows
NCORES = 8
KT = U // 128    # 4 k-tiles
KF = KT * F      # 164 cols per step (k-major hidden layout)
CH = 12          # steps per x chunk
NCH = (S + CH - 1) // CH   # 17 chunks
LAG = 2          # L1 wavefront lag

_CACHE = {}


def _build():
    import os
    steps = int(os.environ.get("BL_STEPS", S))
    do_coll = os.environ.get("BL_COLLECTIVE", "1") == "1"
    dr_mode = os.environ.get("BL_DR", "x")
    dr_x = dr_mode in ("1", "x")
    dr_h = dr_mode in ("1", "h")
    dr_wx1 = dr_h or dr_mode == "wx1"   # L1 input projection (h0 rhs)
    dr_wh = dr_h or dr_mode == "wh"     # recurrent Wh (h rhs)
    do_dr = dr_x or dr_wx1 or dr_wh
    import concourse.bacc as bacc
    import concourse.tile as tile
    import concourse.bass as bass
    from concourse import mybir

    f32 = mybir.dt.float32
    f16 = mybir.dt.float16
    f8 = mybir.dt.float8e4
    AF = mybir.ActivationFunctionType
    xdt = f8 if dr_x else f16
    hdt = f8 if (dr_wx1 or dr_wh) else f16   # h tiles feed both consumers

    nc = bacc.Bacc("TRN2", target_bir_lowering=False, debug=False,
                   num_devices=NCORES)

    xT = nc.dram_tensor("xT", [U, COLS], xdt, kind="ExternalInput")
    Wx0 = nc.dram_tensor("Wx0", [U, G], xdt, kind="ExternalInput")
    whdt = f8 if dr_wh else f16
    wx1dt = f8 if dr_wx1 else f16
    Wh0 = nc.dram_tensor("Wh0", [U, G], whdt, kind="ExternalInput")
    Wx1 = nc.dram_tensor("Wx1", [U, G], wx1dt, kind="ExternalInput")
    Wh1 = nc.dram_tensor("Wh1", [U, G], whdt, kind="ExternalInput")
    bb0d = nc.dram_tensor("bb0", [128, 2 * 656], f8, kind="ExternalInput")
    bb1d = nc.dram_tensor("bb1", [128, 2 * 656], f8, kind="ExternalInput")
    eye8d = nc.dram_tensor("eye8", [128, 256], f8, kind="ExternalInput")
    Wpd = nc.dram_tensor("Wp", [U, U], f16, kind="ExternalInput")
    skipd = nc.dram_tensor("skip", [U, T], f32, kind="ExternalInput")
    eyed = nc.dram_tensor("eye", [128, 128], f16, kind="ExternalInput")
    outd = nc.dram_tensor("out", [U, T], f32, kind="ExternalOutput")

    with ExitStack() as ctx:
        tc = ctx.enter_context(tile.TileContext(nc))
        wpool = ctx.enter_context(tc.tile_pool(name="w", bufs=1))
        h1pool = ctx.enter_context(tc.tile_pool(name="h1sb", bufs=1))
        dram = ctx.enter_context(tc.tile_pool(name="dram", bufs=1, space="DRAM"))

        pid = nc.partition_id()
        parity = pid % 2
        sbase = parity * (S - 1)
        smul = 1 - 2 * parity

        partial_d = dram.tile([U, T], f16)
        gth_d = dram.tile([2 * U, T], f16)

        # ---- persistent weights (only what the tail phase needs)
        def load_w16(pool, src, kt, cols, tag):
            tiles = []
            for k in range(kt):
                t16 = pool.tile([128, cols], f16, tag=f"{tag}{k}",
                                name=f"w_{tag}{k}")
                nc.sync.dma_start(t16[:], src[k * 128:(k + 1) * 128, :])
                tiles.append(t16)
            return tiles

        def load_wpair(pool, src, kt, cols, tag):
            # adjacent k-tile pairs [128, 2*cols] for DoubleRow lhsT planes
            tiles = []
            for t2 in range(kt // 2):
                t16 = pool.tile([128, 2 * cols], f8, tag=f"{tag}p{t2}",
                                name=f"w_{tag}p{t2}")
                for j in range(2):
                    k = 2 * t2 + j
                    nc.sync.dma_start(t16[:, j * cols:(j + 1) * cols],
                                      src[k * 128:(k + 1) * 128, :])
                tiles.append(t16)
            return tiles

        DR = mybir.MatmulPerfMode.DoubleRow

        wp = load_w16(wpool, Wpd, KT, U, "wp")
        eye = wpool.tile([128, 128], f16, tag="eye")
        nc.sync.dma_start(eye[:], eyed[:])
        eye8 = wpool.tile([128, 256], f8, tag="eye8")
        nc.sync.dma_start(eye8[:], eye8d[:])
        eye8v = eye8[:].rearrange("p (j m) -> p j m", j=2)

        # final-layer hidden stream, [128, (s kf)] fp16
        h1_sb = h1pool.tile([128, S * KF], f16, tag="h1sb")
        h1v = h1_sb[:].rearrange("p (s kf) -> p s kf", kf=KF)

        GATE_OF_TILE = ((0, 1), (2, 3))  # psum tile A=(i,f), B=(g,o)

        with tc.tile_pool(name="wrec", bufs=1) as wrec, \
             tc.tile_pool(name="xp", bufs=3) as xp, \
             tc.tile_pool(name="hr", bufs=6) as hp, \
             tc.tile_pool(name="cr", bufs=3) as cp, \
             tc.tile_pool(name="gp", bufs=2) as gp, \
             tc.tile_pool(name="zz", bufs=1) as zz, \
             tc.tile_pool(name="pp", bufs=8, space="PSUM") as pp:

            load_x = load_wpair if dr_x else load_w16
            wx0 = load_x(wrec, Wx0, KT, G, "wx0")
            wh0 = (load_wpair if dr_wh else load_w16)(wrec, Wh0, KT, G, "wh0")
            wx1 = (load_wpair if dr_wx1 else load_w16)(wrec, Wx1, KT, G, "wx1")
            wh1 = (load_wpair if dr_wh else load_w16)(wrec, Wh1, KT, G, "wh1")
            bb0 = wrec.tile([128, 1312], f8, tag="bb0")
            nc.sync.dma_start(bb0[:], bb0d[:])
            bb1 = wrec.tile([128, 1312], f8, tag="bb1")
            nc.sync.dma_start(bb1[:], bb1d[:])

            hz = zz.tile([128, KF], hdt, tag="hz")
            nc.vector.memset(hz[:], 0.0)
            cz0 = zz.tile([128, KF], f32, tag="cz0")
            nc.vector.memset(cz0[:], 0.0)
            cz1 = zz.tile([128, KF], f32, tag="cz1")
            nc.vector.memset(cz1[:], 0.0)

            xchunks = [None] * NCH

            def load_chunk(c):
                n = min(CH, steps - c * CH) * F
                xc = xp.tile([128, KT * CH * F], xdt, tag="xc")
                for k in range(KT):
                    nc.sync.dma_start(
                        xc[:, k * CH * F:k * CH * F + n],
                        xT[k * 128:(k + 1) * 128,
                           c * CH * F:c * CH * F + n])
                xchunks[c] = xc

            def emit_step(layer, s, hprev, cprev, xrhs, wx, wh, bb):
                psA = pp.tile([128, 328], f32, tag="ps")
                psB = pp.tile([128, 328], f32, tag="ps")
                ps = (psA, psB)
                bbv = bb[:].rearrange("p (j n) -> p j n", j=2)
                nc.tensor.matmul(psA[:], eye8v, bbv[:, :, 0:328],
                                 start=True, stop=False, perf_mode=DR,
                                 skip_group_check=True)
                nc.tensor.matmul(psB[:], eye8v, bbv[:, :, 328:656],
                                 start=True, stop=False, perf_mode=DR,
                                 skip_group_check=True)
                def wslc(wt, t2, col):
                    return wt[t2][:].rearrange(
                        "p (j g) -> p j g", j=2)[:, :, col:col + 128]

                def gemm(wtiles, rhs2, rhs1, use_dr, is_last):
                    for ti in range(2):
                        for gi, g in enumerate(GATE_OF_TILE[ti]):
                            for m in range(4):
                                col = g * 512 + m * 128
                                dst = ps[ti][:, gi * 164 + m * F:
                                             gi * 164 + (m + 1) * F]
                                if use_dr:
                                    for t2 in range(KT // 2):
                                        nc.tensor.matmul(
                                            dst, wslc(wtiles, t2, col),
                                            rhs2(t2), start=False,
                                            stop=(is_last and gi == 1
                                                  and m == 3 and t2 == 1),
                                            perf_mode=DR,
                                            skip_group_check=True)
                                else:
                                    for k in range(KT):
                                        nc.tensor.matmul(
                                            dst,
                                            wtiles[k][:, col:col + 128],
                                            rhs1(k), start=False,
                                            stop=(is_last and gi == 1
                                                  and m == 3 and k == KT - 1),
                                            skip_group_check=True)

                # input injection (no recurrent dep), then recurrent part
                gemm(wx, xrhs[0], xrhs[1], xrhs[2], False)
                if dr_wh:
                    hrhs = hprev[:].rearrange(
                        "p (t2 j f) -> p t2 j f", t2=2, j=2)
                    gemm(wh, lambda t2: hrhs[:, t2], None, True, True)
                else:
                    gemm(wh, None,
                         lambda k: hprev[:, k * F:(k + 1) * F], False, True)
                gsc = 1.0 / 16.0 if do_dr else 1.0  # uniform weight scaling
                sig_if = gp.tile([128, 328], f32, tag=f"if{layer}")
                nc.scalar.activation(sig_if[:], psA[:], AF.Sigmoid, scale=gsc)
                g32 = gp.tile([128, KF], f32, tag=f"g{layer}")
                nc.scalar.activation(g32[:], psB[:, 0:164], AF.Tanh, scale=gsc)
                o32 = gp.tile([128, KF], f32, tag=f"o{layer}")
                nc.scalar.activation(o32[:], psB[:, 164:328], AF.Sigmoid,
                                     scale=gsc)
                t1 = gp.tile([128, KF], f32, tag=f"t1{layer}")
                nc.vector.tensor_mul(t1[:], sig_if[:, 0:164], g32[:])
                cnew = cp.tile([128, KF], f32, tag=f"c{layer}")
                nc.vector.tensor_mul(cnew[:], sig_if[:, 164:328], cprev[:])
                nc.vector.tensor_add(cnew[:], cnew[:], t1[:])
                tc32 = gp.tile([128, KF], f32, tag=f"tc{layer}")
                nc.scalar.activation(tc32[:], cnew[:], AF.Tanh)
                hnew = hp.tile([128, KF], hdt, tag=f"h{layer}")
                nc.vector.tensor_mul(hnew[:], o32[:], tc32[:])
                return hnew, cnew

            h0s, c0s = hz, cz0
            h1s, c1s = hz, cz1
            h0bystep = {}
            for w in range(steps + LAG):
                if w < steps:
                    if w % CH == 0:
                        load_chunk(w // CH)
                    xc = xchunks[w // CH]
                    so = (w % CH) * F
                    xcr = xc[:].rearrange(
                        "p (t2 j sf) -> p t2 j sf", t2=2, j=2)
                    xrhs = ((lambda t2, xcr=xcr, so=so:
                             xcr[:, t2, :, so:so + F]),
                            (lambda k, xc=xc, so=so:
                             xc[:, k * CH * F + so:k * CH * F + so + F]),
                            dr_x)
                    h0s, c0s = emit_step(0, w, h0s, c0s, xrhs,
                                         wx0, wh0, bb0)
                    h0bystep[w] = h0s
                if w >= LAG:
                    s = w - LAG
                    h0in = h0bystep.pop(s)
                    h0r = h0in[:].rearrange(
                        "p (t2 j f) -> p t2 j f", t2=2, j=2)
                    xrhs1 = ((lambda t2, h0r=h0r: h0r[:, t2]),
                             (lambda k, h0in=h0in:
                              h0in[:, k * F:(k + 1) * F]),
                             dr_wx1)
                    h1s, c1s = emit_step(1, s, h1s, c1s, xrhs1,
                                         wx1, wh1, bb1)
                    off = nc.s_assert_within(sbase + s * smul, 0, S - 1,
                                             skip_runtime_assert=True)
                    nc.vector.tensor_copy(
                        h1v[:, bass.ds(off, 1), :],
                        h1s[:].rearrange("p (one kf) -> p one kf", one=1))

        # ---- projection + overlap-add (accum seeded with skip + bp)
        tailp = ctx.enter_context(tc.tile_pool(name="tail", bufs=1))
        accum = tailp.tile([128, 4 * T], f32, tag="acc")
        for m in range(4):
            nc.sync.dma_start(accum[:, m * T:(m + 1) * T],
                              skipd[m * 128:(m + 1) * 128, :])

        h1f = h1_sb[:].rearrange("p (s kf) -> p kf s", kf=KF)
        with tc.tile_pool(name="prp", bufs=8, space="PSUM") as ppp:
            for f in range(F):
                for m in range(4):
                    psP = ppp.tile([128, S], f32, tag="pp")
                    for k in range(KT):
                        nc.tensor.matmul(
                            psP[:], wp[k][:, m * 128:(m + 1) * 128],
                            h1f[:, k * F + f, :],
                            start=(k == 0), stop=(k == KT - 1))
                    a0 = m * T + f * STRIDE
                    nc.vector.tensor_add(accum[:, a0:a0 + S],
                                         accum[:, a0:a0 + S], psP[:])

        # ---- pair exchange of projected partials (converted to fp16)
        with tc.tile_pool(name="cvt", bufs=2) as cvt:
            for m in range(4):
                p16 = cvt.tile([128, T], f16, tag="p16")
                nc.vector.tensor_copy(p16[:], accum[:, m * T:(m + 1) * T])
                nc.sync.dma_start(partial_d[m * 128:(m + 1) * 128, :], p16[:])
        if do_coll:
            nc.gpsimd.collective_compute(
                "AllGather", mybir.AluOpType.bypass,
                replica_groups=[[0, 1], [2, 3], [4, 5], [6, 7]],
                ins=[partial_d[:]], outs=[gth_d[:]])
        else:
            nc.sync.dma_start(gth_d[0:U, :], partial_d[:])
            nc.sync.dma_start(gth_d[U:2 * U, :], partial_d[:])

        # add the peer's partial into accum in place, then store
        peer = 1 - parity
        gth_v = gth_d[:].rearrange("(two u) t -> two u t", two=2)
        with tc.tile_pool(name="fin", bufs=2) as fp:
            for m in range(4):
                b = fp.tile([128, T], f16, tag="b")
                nc.sync.dma_start(
                    b[:], gth_v[bass.ds(peer, 1), m * 128:(m + 1) * 128, :])
                a0 = m * T
                nc.vector.tensor_add(accum[:, a0:a0 + T],
                                     accum[:, a0:a0 + T], b[:])
                nc.sync.dma_start(outd[m * 128:(m + 1) * 128, :],
                                  accum[:, a0:a0 + T])

    nc.compile()
    return nc


def _prep_inputs(inputs, Wx_f0, Wh_f0, b_f0, Wx_f1, Wh_f1, b_f1,
                 Wx_b0, Wh_b0, b_b0, Wx_b1, Wh_b1, b_b1, Wp, bp):
    import os
    dr_mode = os.environ.get("BL_DR", "x")
    import ml_dtypes
    f8np = ml_dtypes.float8_e4m3
    xnp = f8np if dr_mode in ("1", "x") else np.float16
    wx1np = f8np if dr_mode in ("1", "h", "wx1") else np.float16
    whnp = f8np if dr_mode in ("1", "h", "wh") else np.float16
    wscale = 16.0 if dr_mode != "0" else 1.0
    x = np.asarray(inputs, dtype=np.float32)  # [4, 512, 4200]
    eye = np.eye(128, dtype=np.float16)
    idx = np.arange(F)[:, None] * STRIDE + np.arange(S)[None, :]  # [F, S]
    wsets = {
        0: (Wx_f0, Wh_f0, b_f0, Wx_f1, Wh_f1, b_f1),
        1: (Wx_b0, Wh_b0, b_b0, Wx_b1, Wh_b1, b_b1),
    }

    def bias_bcast(b):
        # [128, 1312] = [bias(656) | zeros(656)] (DoubleRow zero plane);
        # bias cols [ti*328 + gi*164 + m*41 + j] = b[g*512 + m*128 + p]
        b = (np.asarray(b, np.float32) * wscale).reshape(4, 4, 128)
        out = np.zeros((128, 2 * 656), np.float32)
        for ti, gates in enumerate(((0, 1), (2, 3))):
            for gi, g in enumerate(gates):
                for m in range(4):
                    c0 = ti * 328 + gi * 164 + m * F
                    out[:, c0:c0 + F] = b[g, m][:, None]
        return out.astype(f8np)

    Wp = np.asarray(Wp, np.float32)           # [2U, U]
    bp = np.asarray(bp, np.float32)
    in_maps = []
    for c in range(NCORES):
        q, parity = c // 2, c % 2
        xs = x[q][:, idx]                       # [U, F, S]
        if parity:
            xs = xs[:, :, ::-1]
        xTc = np.ascontiguousarray(
            xs.transpose(0, 2, 1).reshape(U, COLS)).astype(xnp)
        wx0, wh0, b0, wx1, wh1, b1 = wsets[parity]
        if parity == 0:
            sk = (x[q] + bp[:, None]).astype(np.float32)
        else:
            sk = np.zeros((U, T), dtype=np.float32)
        in_maps.append({
            "xT": xTc,
            "Wx0": (np.asarray(wx0, np.float32) * wscale).astype(xnp),
            "Wh0": (np.asarray(wh0, np.float32) * wscale).astype(whnp),
            "Wx1": (np.asarray(wx1, np.float32) * wscale).astype(wx1np),
            "Wh1": (np.asarray(wh1, np.float32) * wscale).astype(whnp),
            "bb0": bias_bcast(b0),
            "bb1": bias_bcast(b1),
            "Wp": Wp[parity * U:(parity + 1) * U, :].astype(np.float16),
            "skip": sk,
            "eye": eye,
            "eye8": np.concatenate(
                [np.eye(128, dtype=np.float32),
                 np.zeros((128, 128), np.float32)], axis=1).astype(f8np),
        })
    return in_maps


def kernel(**inputs) -> np.ndarray:
    from concourse.bass_utils import run_bass_kernel_spmd

    if "nc" not in _CACHE:
        _CACHE["nc"] = _build()
    nc = _CACHE["nc"]

    import os
    in_maps = _prep_inputs(**inputs)
    trace = os.environ.get("BL_TRACE", "0") == "1"
    res = run_bass_kernel_spmd(nc, in_maps, list(range(NCORES)), trace=trace)
    _CACHE["last_result"] = res

    out = np.zeros((4, U, T), dtype=np.float32)
    for q in range(4):
        out[q] = res.results[2 * q]["out"]
    return out


# revision 11
# speedup vs baseline: 1.0251x; 1.0251x over previous
"""BLSTM Trainium2 kernel: 8-core SPMD, SBUF-resident wavefront.

Core pair q={2q,2q+1} owns batch element q; even core runs the forward
2-layer LSTM chain, odd the backward chain (host feeds time-reversed
frames). Both layers are software-pipelined (L1 lags L0 by 2 steps) on
one core; Wx@x_t and the bias are fused into each step's gate PSUM
accumulation, so nothing round-trips through DRAM. The final-layer
hidden stream stays in SBUF (a dynamic-offset copy un-reverses time on
odd cores), is projected per-frame through this core's half of Wp, and
overlap-added into an fp16 accumulator seeded with skip+bias. The pair
exchanges projected partials; both cores emit the identical full
[512, 4200] output (host reads the even core's).
"""
import numpy as np
from contextlib import ExitStack

U = 512
S = 200          # frame width (LSTM steps)
F = 41           # frames per batch element
T = 4200
STRIDE = 100
COLS = S * F     # 8200 time-major columns per core (col = s*41 + f)
G = 4 * U        # 2048 gate rows
NCORES = 8
KT = U // 128    # 4 k-tiles
KF = KT * F      # 164 cols per step (k-major hidden layout)
CH = 12          # steps per x chunk
NCH = (S + CH - 1) // CH   # 17 chunks
LAG = 2          # L1 wavefront lag

_CACHE = {}


def _build():
    import os
    steps = int(os.environ.get("BL_STEPS", S))
    do_coll = os.environ.get("BL_COLLECTIVE", "1") == "1"
    dr_mode = os.environ.get("BL_DR", "x")
    dr_x = dr_mode in ("1", "x")
    dr_h = dr_mode in ("1", "h")
    dr_wx1 = dr_h or dr_mode == "wx1"   # L1 input projection (h0 rhs)
    dr_wh = dr_h or dr_mode == "wh"     # recurrent Wh (h rhs)
    do_dr = dr_x or dr_wx1 or dr_wh
    import concourse.bacc as bacc
    import concourse.tile as tile
    import concourse.bass as bass
    from concourse import mybir

    f32 = mybir.dt.float32
    f16 = mybir.dt.float16
    f8 = mybir.dt.float8e4
    AF = mybir.ActivationFunctionType
    xdt = f8 if dr_x else f16
    hdt = f8 if (dr_wx1 or dr_wh) else f16   # h tiles feed both consumers

    nc = bacc.Bacc("TRN2", target_bir_lowering=False, debug=False,
                   num_devices=NCORES)

    xT = nc.dram_tensor("xT", [U, COLS], xdt, kind="ExternalInput")
    Wx0 = nc.dram_tensor("Wx0", [U, G], xdt, kind="ExternalInput")
    whdt = f8 if dr_wh else f16
    wx1dt = f8 if dr_wx1 else f16
    Wh0 = nc.dram_tensor("Wh0", [U, G], whdt, kind="ExternalInput")
    Wx1 = nc.dram_tensor("Wx1", [U, G], wx1dt, kind="ExternalInput")
    Wh1 = nc.dram_tensor("Wh1", [U, G], whdt, kind="ExternalInput")
    bb0d = nc.dram_tensor("bb0", [128, 2 * 656], f8, kind="ExternalInput")
    bb1d = nc.dram_tensor("bb1", [128, 2 * 656], f8, kind="ExternalInput")
    eye8d = nc.dram_tensor("eye8", [128, 256], f8, kind="ExternalInput")
    Wpd = nc.dram_tensor("Wp", [U, U], f16, kind="ExternalInput")
    skipd = nc.dram_tensor("skip", [U, T], f32, kind="ExternalInput")
    eyed = nc.dram_tensor("eye", [128, 128], f16, kind="ExternalInput")
    outd = nc.dram_tensor("out", [U, T], f16, kind="ExternalOutput")

    with ExitStack() as ctx:
        tc = ctx.enter_context(tile.TileContext(nc))
        wpool = ctx.enter_context(tc.tile_pool(name="w", bufs=1))
        h1pool = ctx.enter_context(tc.tile_pool(name="h1sb", bufs=1))
        dram = ctx.enter_context(tc.tile_pool(name="dram", bufs=1, space="DRAM"))

        pid = nc.partition_id()
        parity = pid % 2
        sbase = parity * (S - 1)
        smul = 1 - 2 * parity

        partial_d = dram.tile([U, T], f16)
        gth_d = dram.tile([2 * U, T], f16)

        # ---- persistent weights (only what the tail phase needs)
        def load_w16(pool, src, kt, cols, tag):
            tiles = []
            for k in range(kt):
                t16 = pool.tile([128, cols], f16, tag=f"{tag}{k}",
                                name=f"w_{tag}{k}")
                nc.sync.dma_start(t16[:], src[k * 128:(k + 1) * 128, :])
                tiles.append(t16)
            return tiles

        def load_wpair(pool, src, kt, cols, tag):
            # adjacent k-tile pairs [128, 2*cols] for DoubleRow lhsT planes
            tiles = []
            for t2 in range(kt // 2):
                t16 = pool.tile([128, 2 * cols], f8, tag=f"{tag}p{t2}",
                                name=f"w_{tag}p{t2}")
                for j in range(2):
                    k = 2 * t2 + j
                    nc.sync.dma_start(t16[:, j * cols:(j + 1) * cols],
                                      src[k * 128:(k + 1) * 128, :])
                tiles.append(t16)
            return tiles

        DR = mybir.MatmulPerfMode.DoubleRow

        wp = load_w16(wpool, Wpd, KT, U, "wp")
        eye = wpool.tile([128, 128], f16, tag="eye")
        nc.sync.dma_start(eye[:], eyed[:])
        eye8 = wpool.tile([128, 256], f8, tag="eye8")
        nc.sync.dma_start(eye8[:], eye8d[:])
        eye8v = eye8[:].rearrange("p (j m) -> p j m", j=2)

        # final-layer hidden stream, [128, (s kf)] fp16
        h1_sb = h1pool.tile([128, S * KF], f16, tag="h1sb")
        h1v = h1_sb[:].rearrange("p (s kf) -> p s kf", kf=KF)

        GATE_OF_TILE = ((0, 1), (2, 3))  # psum tile A=(i,f), B=(g,o)

        with tc.tile_pool(name="wrec", bufs=1) as wrec, \
             tc.tile_pool(name="xp", bufs=3) as xp, \
             tc.tile_pool(name="hr", bufs=6) as hp, \
             tc.tile_pool(name="cr", bufs=3) as cp, \
             tc.tile_pool(name="gp", bufs=2) as gp, \
             tc.tile_pool(name="zz", bufs=1) as zz, \
             tc.tile_pool(name="pp", bufs=8, space="PSUM") as pp:

            load_x = load_wpair if dr_x else load_w16
            wx0 = load_x(wrec, Wx0, KT, G, "wx0")
            wh0 = (load_wpair if dr_wh else load_w16)(wrec, Wh0, KT, G, "wh0")
            wx1 = (load_wpair if dr_wx1 else load_w16)(wrec, Wx1, KT, G, "wx1")
            wh1 = (load_wpair if dr_wh else load_w16)(wrec, Wh1, KT, G, "wh1")
            bb0 = wrec.tile([128, 1312], f8, tag="bb0")
            nc.sync.dma_start(bb0[:], bb0d[:])
            bb1 = wrec.tile([128, 1312], f8, tag="bb1")
            nc.sync.dma_start(bb1[:], bb1d[:])

            hz = zz.tile([128, KF], hdt, tag="hz")
            nc.vector.memset(hz[:], 0.0)
            cz0 = zz.tile([128, KF], f32, tag="cz0")
            nc.vector.memset(cz0[:], 0.0)
            cz1 = zz.tile([128, KF], f32, tag="cz1")
            nc.vector.memset(cz1[:], 0.0)

            xchunks = [None] * NCH

            def load_chunk(c):
                n = min(CH, steps - c * CH) * F
                xc = xp.tile([128, KT * CH * F], xdt, tag="xc")
                for k in range(KT):
                    nc.sync.dma_start(
                        xc[:, k * CH * F:k * CH * F + n],
                        xT[k * 128:(k + 1) * 128,
                           c * CH * F:c * CH * F + n])
                xchunks[c] = xc

            def emit_step(layer, s, hprev, cprev, xrhs, wx, wh, bb):
                psA = pp.tile([128, 328], f32, tag="ps")
                psB = pp.tile([128, 328], f32, tag="ps")
                ps = (psA, psB)
                bbv = bb[:].rearrange("p (j n) -> p j n", j=2)
                nc.tensor.matmul(psA[:], eye8v, bbv[:, :, 0:328],
                                 start=True, stop=False, perf_mode=DR,
                                 skip_group_check=True)
                nc.tensor.matmul(psB[:], eye8v, bbv[:, :, 328:656],
                                 start=True, stop=False, perf_mode=DR,
                                 skip_group_check=True)
                def wslc(wt, t2, col):
                    return wt[t2][:].rearrange(
                        "p (j g) -> p j g", j=2)[:, :, col:col + 128]

                def gemm(wtiles, rhs2, rhs1, use_dr, is_last):
                    for ti in range(2):
                        for gi, g in enumerate(GATE_OF_TILE[ti]):
                            for m in range(4):
                                col = g * 512 + m * 128
                                dst = ps[ti][:, gi * 164 + m * F:
                                             gi * 164 + (m + 1) * F]
                                if use_dr:
                                    for t2 in range(KT // 2):
                                        nc.tensor.matmul(
                                            dst, wslc(wtiles, t2, col),
                                            rhs2(t2), start=False,
                                            stop=(is_last and gi == 1
                                                  and m == 3 and t2 == 1),
                                            perf_mode=DR,
                                            skip_group_check=True)
                                else:
                                    for k in range(KT):
                                        nc.tensor.matmul(
                                            dst,
                                            wtiles[k][:, col:col + 128],
                                            rhs1(k), start=False,
                                            stop=(is_last and gi == 1
                                                  and m == 3 and k == KT - 1),
                                            skip_group_check=True)

                # input injection (no recurrent dep), then recurrent part
                gemm(wx, xrhs[0], xrhs[1], xrhs[2], False)
                if dr_wh:
                    hrhs = hprev[:].rearrange(
                        "p (t2 j f) -> p t2 j f", t2=2, j=2)
                    gemm(wh, lambda t2: hrhs[:, t2], None, True, True)
                else:
                    gemm(wh, None,
                         lambda k: hprev[:, k * F:(k + 1) * F], False, True)
                gsc = 1.0 / 16.0 if do_dr else 1.0  # uniform weight scaling
                sig_if = gp.tile([128, 328], f32, tag=f"if{layer}")
                nc.scalar.activation(sig_if[:], psA[:], AF.Sigmoid, scale=gsc)
                g32 = gp.tile([128, KF], f32, tag=f"g{layer}")
                nc.scalar.activation(g32[:], psB[:, 0:164], AF.Tanh, scale=gsc)
                o32 = gp.tile([128, KF], f32, tag=f"o{layer}")
                nc.scalar.activation(o32[:], psB[:, 164:328], AF.Sigmoid,
                                     scale=gsc)
                t1 = gp.tile([128, KF], f32, tag=f"t1{layer}")
                nc.vector.tensor_mul(t1[:], sig_if[:, 0:164], g32[:])
                cnew = cp.tile([128, KF], f32, tag=f"c{layer}")
                nc.vector.tensor_mul(cnew[:], sig_if[:, 164:328], cprev[:])
                nc.vector.tensor_add(cnew[:], cnew[:], t1[:])
                tc32 = gp.tile([128, KF], f32, tag=f"tc{layer}")
                nc.scalar.activation(tc32[:], cnew[:], AF.Tanh)
                hnew = hp.tile([128, KF], hdt, tag=f"h{layer}")
                nc.vector.tensor_mul(hnew[:], o32[:], tc32[:])
                return hnew, cnew

            h0s, c0s = hz, cz0
            h1s, c1s = hz, cz1
            h0bystep = {}
            for w in range(steps + LAG):
                if w < steps:
                    if w % CH == 0:
                        load_chunk(w // CH)
                    xc = xchunks[w // CH]
                    so = (w % CH) * F
                    xcr = xc[:].rearrange(
                        "p (t2 j sf) -> p t2 j sf", t2=2, j=2)
                    xrhs = ((lambda t2, xcr=xcr, so=so:
                             xcr[:, t2, :, so:so + F]),
                            (lambda k, xc=xc, so=so:
                             xc[:, k * CH * F + so:k * CH * F + so + F]),
                            dr_x)
                    h0s, c0s = emit_step(0, w, h0s, c0s, xrhs,
                                         wx0, wh0, bb0)
                    h0bystep[w] = h0s
                if w >= LAG:
                    s = w - LAG
                    h0in = h0bystep.pop(s)
                    h0r = h0in[:].rearrange(
                        "p (t2 j f) -> p t2 j f", t2=2, j=2)
                    xrhs1 = ((lambda t2, h0r=h0r: h0r[:, t2]),
                             (lambda k, h0in=h0in:
                              h0in[:, k * F:(k + 1) * F]),
                             dr_wx1)
                    h1s, c1s = emit_step(1, s, h1s, c1s, xrhs1,
                                         wx1, wh1, bb1)
                    off = nc.s_assert_within(sbase + s * smul, 0, S - 1,
                                             skip_runtime_assert=True)
                    nc.vector.tensor_copy(
                        h1v[:, bass.ds(off, 1), :],
                        h1s[:].rearrange("p (one kf) -> p one kf", one=1))

        # ---- projection + overlap-add (accum seeded with skip + bp)
        tailp = ctx.enter_context(tc.tile_pool(name="tail", bufs=1))
        accum = tailp.tile([128, 4 * T], f32, tag="acc")
        for m in range(4):
            nc.sync.dma_start(accum[:, m * T:(m + 1) * T],
                              skipd[m * 128:(m + 1) * 128, :])

        h1f = h1_sb[:].rearrange("p (s kf) -> p kf s", kf=KF)
        with tc.tile_pool(name="prp", bufs=8, space="PSUM") as ppp:
            for f in range(F):
                for m in range(4):
                    psP = ppp.tile([128, S], f32, tag="pp")
                    for k in range(KT):
                        nc.tensor.matmul(
                            psP[:], wp[k][:, m * 128:(m + 1) * 128],
                            h1f[:, k * F + f, :],
                            start=(k == 0), stop=(k == KT - 1))
                    a0 = m * T + f * STRIDE
                    nc.vector.tensor_add(accum[:, a0:a0 + S],
                                         accum[:, a0:a0 + S], psP[:])

        # ---- pair exchange of projected partials (converted to fp16)
        with tc.tile_pool(name="cvt", bufs=2) as cvt:
            for m in range(4):
                p16 = cvt.tile([128, T], f16, tag="p16")
                nc.vector.tensor_copy(p16[:], accum[:, m * T:(m + 1) * T])
                nc.sync.dma_start(partial_d[m * 128:(m + 1) * 128, :], p16[:])
        if do_coll:
            nc.gpsimd.collective_compute(
                "AllGather", mybir.AluOpType.bypass,
                replica_groups=[[0, 1], [2, 3], [4, 5], [6, 7]],
                ins=[partial_d[:]], outs=[gth_d[:]])
        else:
            nc.sync.dma_start(gth_d[U:2 * U, :], partial_d[:])

        # add the peer's partial into accum in place, then store
        peer = 1 - parity
        gth_v = gth_d[:].rearrange("(two u) t -> two u t", two=2)
        with tc.tile_pool(name="fin", bufs=2) as fp:
            for m in range(4):
                b = fp.tile([128, T], f16, tag="b")
                nc.sync.dma_start(
                    b[:], gth_v[bass.ds(peer, 1), m * 128:(m + 1) * 128, :])
                o16 = fp.tile([128, T], f16, tag="o16")
                nc.vector.tensor_add(o16[:], accum[:, m * T:(m + 1) * T],
                                     b[:])
                nc.sync.dma_start(outd[m * 128:(m + 1) * 128, :], o16[:])

    nc.compile()
    return nc


def _prep_inputs(inputs, Wx_f0, Wh_f0, b_f0, Wx_f1, Wh_f1, b_f1,
                 Wx_b0, Wh_b0, b_b0, Wx_b1, Wh_b1, b_b1, Wp, bp):
    import os
    dr_mode = os.environ.get("BL_DR", "x")
    import ml_dtypes
    f8np = ml_dtypes.float8_e4m3
    xnp = f8np if dr_mode in ("1", "x") else np.float16
    wx1np = f8np if dr_mode in ("1", "h", "wx1") else np.float16
    whnp = f8np if dr_mode in ("1", "h", "wh") else np.float16
    wscale = 16.0 if dr_mode != "0" else 1.0
    x = np.asarray(inputs, dtype=np.float32)  # [4, 512, 4200]
    eye = np.eye(128, dtype=np.float16)
    idx = np.arange(F)[:, None] * STRIDE + np.arange(S)[None, :]  # [F, S]
    wsets = {
        0: (Wx_f0, Wh_f0, b_f0, Wx_f1, Wh_f1, b_f1),
        1: (Wx_b0, Wh_b0, b_b0, Wx_b1, Wh_b1, b_b1),
    }

    def bias_bcast(b):
        # [128, 1312] = [bias(656) | zeros(656)] (DoubleRow zero plane);
        # bias cols [ti*328 + gi*164 + m*41 + j] = b[g*512 + m*128 + p]
        b = (np.asarray(b, np.float32) * wscale).reshape(4, 4, 128)
        out = np.zeros((128, 2 * 656), np.float32)
        for ti, gates in enumerate(((0, 1), (2, 3))):
            for gi, g in enumerate(gates):
                for m in range(4):
                    c0 = ti * 328 + gi * 164 + m * F
                    out[:, c0:c0 + F] = b[g, m][:, None]
        return out.astype(f8np)

    Wp = np.asarray(Wp, np.float32)           # [2U, U]
    bp = np.asarray(bp, np.float32)
    in_maps = []
    for c in range(NCORES):
        q, parity = c // 2, c % 2
        xs = x[q][:, idx]                       # [U, F, S]
        if parity:
            xs = xs[:, :, ::-1]
        xTc = np.ascontiguousarray(
            xs.transpose(0, 2, 1).reshape(U, COLS)).astype(xnp)
        wx0, wh0, b0, wx1, wh1, b1 = wsets[parity]
        if parity == 0:
            sk = (x[q] + bp[:, None]).astype(np.float32)
        else:
            sk = np.zeros((U, T), dtype=np.float32)
        in_maps.append({
            "xT": xTc,
            "Wx0": (np.asarray(wx0, np.float32) * wscale).astype(xnp),
            "Wh0": (np.asarray(wh0, np.float32) * wscale).astype(whnp),
            "Wx1": (np.asarray(wx1, np.float32) * wscale).astype(wx1np),
            "Wh1": (np.asarray(wh1, np.float32) * wscale).astype(whnp),
            "bb0": bias_bcast(b0),
            "bb1": bias_bcast(b1),
            "Wp": Wp[parity * U:(parity + 1) * U, :].astype(np.float16),
            "skip": sk,
            "eye": eye,
            "eye8": np.concatenate(
                [np.eye(128, dtype=np.float32),
                 np.zeros((128, 128), np.float32)], axis=1).astype(f8np),
        })
    return in_maps


def kernel(**inputs) -> np.ndarray:
    from concourse.bass_utils import run_bass_kernel_spmd

    if "nc" not in _CACHE:
        _CACHE["nc"] = _build()
    nc = _CACHE["nc"]

    import os
    in_maps = _prep_inputs(**inputs)
    trace = os.environ.get("BL_TRACE", "0") == "1"
    res = run_bass_kernel_spmd(nc, in_maps, list(range(NCORES)), trace=trace)
    _CACHE["last_result"] = res

    out = np.zeros((4, U, T), dtype=np.float32)
    for q in range(4):
        out[q] = res.results[2 * q]["out"]
    return out
